# revision 5
# baseline (speedup 1.0000x reference)
"""GridMask kernel for Trainium2 (8 NeuronCores, batch-sharded SPMD).

out[n,c,s,h,w] = x[n,c,s,h,w] * mask[n,s,h,w]
mask = row_hit OR col_hit, where row_hit/col_hit are per-(n,s) stripe
predicates on h / w respectively.

Strategy:
  - Host computes the tiny per-(n,s) row/col stripe-hit vectors ([S,H] and
    [S,W] floats per batch element) from d/st_h/st_w.
  - Each of the 8 cores processes one batch element n (50.3MB in + 50.3MB out).
  - On-chip, the [128,512] mask tile for a row-chunk is built by the (idle)
    TensorEngine as a K=3 outer product into PSUM:
        mask = row*1 + 1*col + (-row)*col = row OR col   (values in {0,1})
  - The VectorEngine multiplies x tiles (SBUF) by mask (PSUM) in place.
  - DMAs are 1MB each ([128, 2048] f32 = one (c,s) [512,512] slab), loads on
    the SP HWDGE ring, stores on the ACT HWDGE ring. Rows are chunked as
    h = p*4 + k (partition-outer), which makes every DMA a single fully
    contiguous 8KB run per partition (2-dim AP) — measured ~443 GB/s
    sustained vs ~400 GB/s for the strided h = k*128 + p layout.
"""

import math

import numpy as np

# problem shapes (hardcoded per harness contract)
N, C, S, H, W = 8, 3, 16, 512, 512
RATIO = 0.5
HH = math.ceil(math.sqrt(H * H + W * W))
OFF_H = (HH - H) // 2
OFF_W = (HH - W) // 2
P = 128
K = H // P  # row chunks per slab
NCORES = 8

_compiled = None


def _build():
    import concourse.bacc as bacc
    import concourse.mybir as mybir
    from concourse.mybir import AluOpType
    from concourse.tile import TileContext

    nc = bacc.Bacc()
    x = nc.dram_tensor("x", [C, S, H, W], mybir.dt.float32, kind="ExternalInput")
    lhsT = nc.dram_tensor("lhsT", [3, S * H], mybir.dt.bfloat16, kind="ExternalInput")
    rhs = nc.dram_tensor("rhs", [3, S * W], mybir.dt.bfloat16, kind="ExternalInput")
    out = nc.dram_tensor("out", [C, S, H, W], mybir.dt.float32, kind="ExternalOutput")

    with TileContext(nc) as tc:
        with (
            tc.tile_pool(name="params", bufs=1) as params,
            tc.tile_pool(name="xp", bufs=6) as xp,
            tc.tile_pool(name="mp", bufs=8, space="PSUM") as mp,
        ):
            lhsT_sb = params.tile([3, S * H], mybir.dt.bfloat16)
            rhs_sb = params.tile([3, S * W], mybir.dt.bfloat16)
            nc.sync.dma_start(out=lhsT_sb[:], in_=lhsT[:, :])
            nc.sync.dma_start(out=rhs_sb[:], in_=rhs[:, :])
            for s in range(S):
                masks = []
                for k in range(K):
                    pm = mp.tile([P, W], mybir.dt.float32)
                    nc.tensor.matmul(
                        pm[:],
                        lhsT_sb[:, s * H + k * P : s * H + (k + 1) * P],
                        rhs_sb[:, s * W : (s + 1) * W],
                        start=True,
                        stop=True,
                    )
                    masks.append(pm)
                for c in range(C):
                    xt = xp.tile([P, K, W], mybir.dt.float32)
                    nc.sync.dma_start(
                        out=xt[:], in_=x[c, s].rearrange("(p k) w -> p k w", p=P)
                    )
                    for k in range(K):
                        nc.vector.tensor_tensor(
                            xt[:, k, :],
                            xt[:, k, :],
                            masks[k][:],
                            AluOpType.mult,
                        )
                    nc.scalar.dma_start(
                        out=out[c, s].rearrange("(p k) w -> p k w", p=P), in_=xt[:]
                    )
    nc.compile()
    return nc


def _hit_vectors(d, st_h, st_w):
    """row_hit [N,S,H] and col_hit [N,S,W] as float32 {0,1}."""
    d3 = d.astype(np.int64)[:, None, None]  # [N,1,1]
    l3 = np.ceil(d.astype(np.float32) * RATIO).astype(np.int64)[:, None, None]
    sth = st_h.astype(np.int64) % d3[:, :, 0]  # [N,S]
    stw = st_w.astype(np.int64) % d3[:, :, 0]
    rr = np.arange(H, dtype=np.int64)
    cc = np.arange(W, dtype=np.int64)
    row_hit = ((rr[None, None, :] + OFF_H - sth[:, :, None]) % d3) < l3
    col_hit = ((cc[None, None, :] + OFF_W - stw[:, :, None]) % d3) < l3
    return row_hit.astype(np.float32), col_hit.astype(np.float32)


def _prep_in_maps(x, d, st_h, st_w):
    import ml_dtypes

    x = np.asarray(x, dtype=np.float32)
    d = np.asarray(d)
    st_h = np.asarray(st_h)
    st_w = np.asarray(st_w)
    rowf, colf = _hit_vectors(d, st_h, st_w)  # [N,S,H], [N,S,W]
    # rows chunked partition-outer: h = p*K + k, so the [3,128] lhsT slice for
    # (s, k) must hold row_hit[s, p*K + k] at free position p
    rowpk = rowf.reshape(N, S, P, K).transpose(0, 1, 3, 2).reshape(N, S * H)
    ones_h = np.ones_like(rowpk)
    ones_w = np.ones_like(colf)
    # lhsT rows: [row, 1, -row]; rhs rows: [1, col, col]
    # => mask = row*1 + 1*col + (-row)*col = row OR col
    lhsT = np.stack([rowpk, ones_h, -rowpk], axis=1).reshape(N, 3, S * H)
    rhs = np.stack([ones_w, colf, colf], axis=1).reshape(N, 3, S * W)
    lhsT = lhsT.astype(ml_dtypes.bfloat16)  # exact for {0, +-1}
    rhs = rhs.astype(ml_dtypes.bfloat16)
    return [
        {
            "x": np.ascontiguousarray(x[n]),
            "lhsT": np.ascontiguousarray(lhsT[n]),
            "rhs": np.ascontiguousarray(rhs[n]),
        }
        for n in range(N)
    ]


def kernel(x, d, st_h, st_w):
    from concourse.bass_utils import run_bass_kernel_spmd

    global _compiled
    if _compiled is None:
        _compiled = _build()
    in_maps = _prep_in_maps(x, d, st_h, st_w)
    res = run_bass_kernel_spmd(_compiled, in_maps, core_ids=list(range(NCORES)))
    return np.stack([r["out"] for r in res.results], axis=0)


# revision 6
# speedup vs baseline: 1.1326x; 1.1326x over previous
"""GridMask kernel for Trainium2 (8 NeuronCores, batch-sharded SPMD).

out[n,c,s,h,w] = x[n,c,s,h,w] * mask[n,s,h,w]
mask = row_hit OR col_hit, where row_hit/col_hit are per-(n,s) stripe
predicates on h / w respectively.

Strategy:
  - Host computes the tiny per-(n,s) row/col stripe-hit vectors ([S,H] and
    [S,W] floats per batch element) from d/st_h/st_w.
  - Each of the 8 cores processes one batch element n (50.3MB in + 50.3MB out).
  - On-chip, the [128,512] mask tile for a row-chunk is built by the (idle)
    TensorEngine as a K=3 outer product into PSUM:
        mask = row*1 + 1*col + (-row)*col = row OR col   (values in {0,1})
  - The VectorEngine multiplies x tiles (SBUF) by mask (PSUM) in place.
  - DMAs are 1MB each ([128, 2048] f32 = one (c,s) [512,512] slab), loads on
    the SP HWDGE ring, stores on the ACT HWDGE ring. Rows are chunked as
    h = p*4 + k (partition-outer), which makes every DMA a single fully
    contiguous 8KB run per partition (2-dim AP) — measured ~443 GB/s
    sustained vs ~400 GB/s for the strided h = k*128 + p layout.
"""

import math

import numpy as np

# problem shapes (hardcoded per harness contract)
N, C, S, H, W = 8, 3, 16, 512, 512
RATIO = 0.5
HH = math.ceil(math.sqrt(H * H + W * W))
OFF_H = (HH - H) // 2
OFF_W = (HH - W) // 2
P = 128
K = H // P  # row chunks per slab
NCORES = 8

_compiled = None


def _build():
    import concourse.bacc as bacc
    import concourse.mybir as mybir
    from concourse.mybir import AluOpType
    from concourse.tile import TileContext

    nc = bacc.Bacc()
    x = nc.dram_tensor("x", [C, S, H, W], mybir.dt.float32, kind="ExternalInput")
    lhsT = nc.dram_tensor("lhsT", [3, S * H], mybir.dt.bfloat16, kind="ExternalInput")
    rhs = nc.dram_tensor("rhs", [3, S * W], mybir.dt.bfloat16, kind="ExternalInput")
    out = nc.dram_tensor("out", [C, S, H, W], mybir.dt.float32, kind="ExternalOutput")

    with TileContext(nc) as tc:
        with (
            tc.tile_pool(name="params", bufs=1) as params,
            tc.tile_pool(name="xp", bufs=6) as xp,
            tc.tile_pool(name="mp", bufs=8, space="PSUM") as mp,
        ):
            lhsT_sb = params.tile([3, S * H], mybir.dt.bfloat16)
            rhs_sb = params.tile([3, S * W], mybir.dt.bfloat16)
            nc.sync.dma_start(out=lhsT_sb[:], in_=lhsT[:, :])
            nc.sync.dma_start(out=rhs_sb[:], in_=rhs[:, :])
            for s in range(S):
                # 4-bank PSUM mask tile for this s; one matmul per bank
                pm = mp.tile([P, K, W], mybir.dt.float32, bufs=2)
                for k in range(K):
                    nc.tensor.matmul(
                        pm[:, k, :],
                        lhsT_sb[:, s * H + k * P : s * H + (k + 1) * P],
                        rhs_sb[:, s * W : (s + 1) * W],
                        start=True,
                        stop=True,
                    )
                for c in range(C):
                    xt = xp.tile([P, K, W], mybir.dt.float32)
                    nc.sync.dma_start(
                        out=xt[:], in_=x[c, s].rearrange("(p k) w -> p k w", p=P)
                    )
                    nc.vector.tensor_tensor(
                        xt[:, :, :], xt[:, :, :], pm[:, :, :], AluOpType.mult
                    )
                    nc.scalar.dma_start(
                        out=out[c, s].rearrange("(p k) w -> p k w", p=P), in_=xt[:]
                    )
    nc.compile()
    return nc


def _hit_vectors(d, st_h, st_w):
    """row_hit [N,S,H] and col_hit [N,S,W] as float32 {0,1}."""
    d3 = d.astype(np.int64)[:, None, None]  # [N,1,1]
    l3 = np.ceil(d.astype(np.float32) * RATIO).astype(np.int64)[:, None, None]
    sth = st_h.astype(np.int64) % d3[:, :, 0]  # [N,S]
    stw = st_w.astype(np.int64) % d3[:, :, 0]
    rr = np.arange(H, dtype=np.int64)
    cc = np.arange(W, dtype=np.int64)
    row_hit = ((rr[None, None, :] + OFF_H - sth[:, :, None]) % d3) < l3
    col_hit = ((cc[None, None, :] + OFF_W - stw[:, :, None]) % d3) < l3
    return row_hit.astype(np.float32), col_hit.astype(np.float32)


def _prep_in_maps(x, d, st_h, st_w):
    import ml_dtypes

    x = np.asarray(x, dtype=np.float32)
    d = np.asarray(d)
    st_h = np.asarray(st_h)
    st_w = np.asarray(st_w)
    rowf, colf = _hit_vectors(d, st_h, st_w)  # [N,S,H], [N,S,W]
    # rows chunked partition-outer: h = p*K + k, so the [3,128] lhsT slice for
    # (s, k) must hold row_hit[s, p*K + k] at free position p
    rowpk = rowf.reshape(N, S, P, K).transpose(0, 1, 3, 2).reshape(N, S * H)
    ones_h = np.ones_like(rowpk)
    ones_w = np.ones_like(colf)
    # lhsT rows: [row, 1, -row]; rhs rows: [1, col, col]
    # => mask = row*1 + 1*col + (-row)*col = row OR col
    lhsT = np.stack([rowpk, ones_h, -rowpk], axis=1).reshape(N, 3, S * H)
    rhs = np.stack([ones_w, colf, colf], axis=1).reshape(N, 3, S * W)
    lhsT = lhsT.astype(ml_dtypes.bfloat16)  # exact for {0, +-1}
    rhs = rhs.astype(ml_dtypes.bfloat16)
    return [
        {
            "x": np.ascontiguousarray(x[n]),
            "lhsT": np.ascontiguousarray(lhsT[n]),
            "rhs": np.ascontiguousarray(rhs[n]),
        }
        for n in range(N)
    ]


def kernel(x, d, st_h, st_w):
    from concourse.bass_utils import run_bass_kernel_spmd

    global _compiled
    if _compiled is None:
        _compiled = _build()
    in_maps = _prep_in_maps(x, d, st_h, st_w)
    res = run_bass_kernel_spmd(_compiled, in_maps, core_ids=list(range(NCORES)))
    return np.stack([r["out"] for r in res.results], axis=0)
